# revision 1
# baseline (speedup 1.0000x reference)
"""GCN (2-layer) Trainium2 kernel over 8 NeuronCores — v2.

Structure per core (dst-shard = 6250 nodes = 49 tiles of 128):
- head: Tsh = dinv * (x @ W1) as bf16 table rows [node, 128] (64 feats + 64
  zero pad so gather rows are 256B); AllGather -> full [50176, 128] bf16
  table in DRAM.
- aggregation: edges grouped per (dst tile t, src half h) into runs padded
  to a cross-core-uniform B[t,h]*128 slots. Per run: one SWDGE dma_gather
  (trailing -1 idxs are skipped by ucode => no Pool cost for padding) and
  B matmuls psum[t] += S_piece^T @ G_piece with HOST-precomputed bf16
  one-hot S pieces streamed from DRAM (no DVE one-hot builds).
- self loops: psum[t] starts with Identity^T @ Tsh[:,t] (no gather).
- tails: layer1 relu/scale -> T2sh table; layer2 transpose + W2 + bias +
  log_softmax.
All 49 psum accumulators live in PSUM simultaneously (7 banks x 8 tiles).
"""

import numpy as np

N_NODES = 50000
CORES = 8
SH = 6250          # owned nodes per core
SHP = 6272         # padded shard rows (49*128)
NT = 49            # dst tiles per core
HALF = SHP * 4     # 25088 table rows per half
F0, F1, F2 = 96, 64, 16
FP = 128           # padded feature width (bf16 row = 256B)
BLK = 128
# Pad slots gather local row 6250 of a shard (zero rows in both halves).
# (-1 trailing-skip saves no Q7 time: the idx-unpack loop dominates, and
# skipped slots leave stale SBUF that poisons the S-matmul with NaNs.)
PAD_IDX = SH


def _row_of_node(n):
    s = n // SH
    return s * SHP + (n - s * SH)


def host_prep(x, edge_index, W1, b1, W2, b2):
    import ml_dtypes
    bf16 = ml_dtypes.bfloat16

    src = np.asarray(edge_index[0], dtype=np.int64)
    dst = np.asarray(edge_index[1], dtype=np.int64)
    deg_full = np.bincount(dst, minlength=N_NODES).astype(np.float32) + 1.0

    # per-core edge lists (dst-sharded), NO self loops (identity pieces)
    order = np.argsort(dst, kind="stable")
    s_sorted, d_sorted = src[order], dst[order]
    bounds = np.searchsorted(d_sorted, np.arange(0, N_NODES + 1, SH))
    runs = [[[None] * 2 for _ in range(NT)] for _ in range(CORES)]
    counts = np.zeros((CORES, NT, 2), dtype=np.int64)
    for i in range(CORES):
        es = s_sorted[bounds[i]:bounds[i + 1]]
        ed = d_sorted[bounds[i]:bounds[i + 1]] - SH * i   # local dst [0,6250)
        rows = _row_of_node(es)
        h = rows // HALF
        lrow = rows - h * HALF
        tile = ed // BLK
        dl = ed - tile * BLK
        key = tile * 2 + h
        o = np.argsort(key, kind="stable")
        key_s, lrow_s, dl_s = key[o], lrow[o], dl[o]
        kb = np.searchsorted(key_s, np.arange(NT * 2 + 1))
        for t in range(NT):
            for hh in (0, 1):
                a, b = kb[t * 2 + hh], kb[t * 2 + hh + 1]
                runs[i][t][hh] = (lrow_s[a:b], dl_s[a:b])
                counts[i, t, hh] = b - a

    B = np.maximum(1, -(-counts.max(axis=0) // BLK))      # [NT, 2] uniform
    half_len = [int(B[:, h].sum()) * BLK for h in (0, 1)]
    npieces = int(B.sum())

    data = []
    for i in range(CORES):
        idx_planes = []
        sblob = np.zeros((BLK, npieces * BLK), dtype=bf16)
        poff = 0
        for h in (0, 1):
            stream = np.empty(0, dtype=np.int64)
            for t in range(NT):
                lr, dl = runs[i][t][h]
                nb = int(B[t, h])
                pad = nb * BLK - len(lr)
                stream = np.concatenate(
                    [stream, lr, np.full(pad, PAD_IDX, np.int64)])
                if h == 0:
                    pass
            idx_planes.append(stream)
        # S blob in PROGRAM ORDER: for t, for h, pieces b
        for t in range(NT):
            for h in (0, 1):
                lr, dl = runs[i][t][h]
                nb = int(B[t, h])
                S = np.zeros((nb * BLK, BLK), np.float32)
                S[np.arange(len(dl)), dl] = 1.0
                Sp = S.reshape(nb, BLK, BLK)
                for b in range(nb):
                    sblob[:, (poff + b) * BLK:(poff + b + 1) * BLK] = \
                        Sp[b].astype(bf16)
                poff += nb
        assert poff == npieces

        planes = []
        for h in (0, 1):
            si = idx_planes[h]
            assert len(si) == half_len[h]
            pl = si.reshape(-1, 16).T.astype(np.int16)    # [16, len/16]
            planes.append(np.tile(pl, (8, 1)))

        degp = np.ones((BLK, NT), np.float32)
        dshard = deg_full[SH * i:SH * (i + 1)]
        dp = np.concatenate([dshard, np.ones(SHP - SH, np.float32)])
        degp[:, :] = dp.reshape(NT, BLK).T

        xs = np.zeros((F0, SHP), np.float32)
        xs[:, :SH] = np.asarray(x[SH * i:SH * (i + 1)], np.float32).T
        data.append(dict(
            xT=np.ascontiguousarray(xs.astype(bf16)),
            idx0=np.ascontiguousarray(planes[0]),
            idx1=np.ascontiguousarray(planes[1]),
            sblob=np.ascontiguousarray(sblob),
            deg=np.ascontiguousarray(degp),
        ))

    consts = dict(
        W1=np.asarray(W1, np.float32).astype(bf16),
        W2=np.asarray(W2, np.float32).astype(bf16),
        b1b=np.tile(np.asarray(b1, np.float32), (BLK, 1)),
        b2b=np.tile(np.asarray(b2, np.float32), (BLK, 1)),
        identb=np.eye(BLK, dtype=np.float32).astype(bf16),
        identf=np.eye(BLK, dtype=np.float32),
    )
    meta = dict(B=B, half_len=half_len, npieces=npieces)
    return data, consts, meta


def numpy_device_sim(data, consts, meta):
    """Replay the device algorithm in numpy (validates idx/S construction)."""
    B = meta["B"]
    npieces = meta["npieces"]
    dinvs, tables = [], []
    for i in range(CORES):
        d = data[i]
        dinv = 1.0 / np.sqrt(d["deg"])                    # [128, NT]
        dinvs.append(dinv)
        h = d["xT"].astype(np.float32).T @ consts["W1"].astype(np.float32)
        hs = h.reshape(NT, BLK, F1) * dinv.T[:, :, None]
        tables.append(hs.reshape(SHP, F1))
    table = np.concatenate(tables, 0)                     # [50176, 64]

    def layer(table, i, d, own):
        halves = [table[:HALF], table[HALF:]]
        agg = np.zeros((NT, BLK, F1), np.float32)
        for t in range(NT):
            agg[t] = own[t]                               # self loop piece
        streams = [(d["idx0"] if h == 0 else d["idx1"])[:16].T.reshape(-1)
                   for h in (0, 1)]
        poff = 0
        soff = [0, 0]
        for t in range(NT):
            for h in (0, 1):
                nb = int(B[t, h])
                for b in range(nb):
                    sl = streams[h][soff[h] + b * BLK: soff[h] + (b + 1) * BLK]
                    G = halves[h][sl.astype(np.int64)]
                    S = d["sblob"][:, (poff + b) * BLK:(poff + b + 1) * BLK]
                    agg[t] += S.astype(np.float32).T @ G
                soff[h] += nb * BLK
                poff += nb
        return agg

    full2 = []
    for i in range(CORES):
        d = data[i]
        own = tables[i].reshape(NT, BLK, F1)
        agg = layer(table, i, d, own)
        dinv = dinvs[i]
        t2 = []
        for t in range(NT):
            e = np.maximum(agg[t] * dinv[:, t:t + 1] + consts["b1b"], 0.0) \
                * dinv[:, t:t + 1]
            t2.append(e)
        full2.append(np.stack(t2).reshape(SHP, F1))
    table2 = np.concatenate(full2, 0)

    outs = []
    for i in range(CORES):
        d = data[i]
        own2 = full2[i].reshape(NT, BLK, F1)
        agg = layer(table2, i, d, own2)
        dinv = dinvs[i]
        o = np.zeros((NT, BLK, F2), np.float32)
        for t in range(NT):
            a = agg[t] * dinv[:, t:t + 1]
            z = a @ consts["W2"].astype(np.float32) + consts["b2b"]
            m = z.max(1, keepdims=True)
            ls = z - m - np.log(np.exp(z - m).sum(1, keepdims=True))
            o[t] = ls
        outs.append(o.reshape(SHP, F2))
    return np.stack(outs)


def assemble_output(outs):
    res = np.zeros((N_NODES, F2), np.float32)
    for i in range(CORES):
        res[SH * i:SH * (i + 1)] = outs[i][:SH]
    return res


def build_nc(meta):
    import concourse.bacc as bacc
    import concourse.tile as tile
    import concourse.mybir as mybir

    dt = mybir.dt
    Alu = mybir.AluOpType
    Act = mybir.ActivationFunctionType
    B = meta["B"]
    half_len = meta["half_len"]
    npieces = meta["npieces"]

    nc = bacc.Bacc(None, target_bir_lowering=False)
    p_xT = nc.declare_dram_parameter("xT", [F0, SHP], dt.bfloat16, isOutput=False)
    p_idx = [nc.declare_dram_parameter(f"idx{h}", [128, half_len[h] // 16],
                                       dt.int16, isOutput=False) for h in (0, 1)]
    p_S = nc.declare_dram_parameter("sblob", [128, npieces * BLK], dt.bfloat16,
                                    isOutput=False)
    p_deg = nc.declare_dram_parameter("deg", [128, NT], dt.float32, isOutput=False)
    p_W1 = nc.declare_dram_parameter("W1", [F0, F1], dt.bfloat16, isOutput=False)
    p_W2 = nc.declare_dram_parameter("W2", [F1, F2], dt.bfloat16, isOutput=False)
    p_b1 = nc.declare_dram_parameter("b1b", [128, F1], dt.float32, isOutput=False)
    p_b2 = nc.declare_dram_parameter("b2b", [128, F2], dt.float32, isOutput=False)
    p_ib = nc.declare_dram_parameter("identb", [128, 128], dt.bfloat16,
                                     isOutput=False)
    p_if = nc.declare_dram_parameter("identf", [128, 128], dt.float32,
                                     isOutput=False)
    p_out = nc.declare_dram_parameter("out", [128, NT * F2], dt.float32,
                                      isOutput=True)

    cc_in = [nc.dram_tensor(f"cc_in{li}", [SHP, FP], dt.bfloat16) for li in (0, 1)]
    cc_out = [nc.dram_tensor(f"cc_out{li}", [CORES * SHP, FP], dt.bfloat16,
                             addr_space="Shared") for li in (0, 1)]

    BMAX = int(B.max())

    with tile.TileContext(nc) as tc:
        with (
            tc.tile_pool(name="cpool", bufs=1) as cpool,
            tc.tile_pool(name="stpool", bufs=8) as stpool,
            tc.tile_pool(name="spool", bufs=8) as spool,
            tc.tile_pool(name="wpool", bufs=4) as wpool,
            tc.tile_pool(name="apool", bufs=4, space="PSUM") as apool,
            tc.tile_pool(name="hpool", bufs=2, space="PSUM") as hpool,
            tc.tile_pool(name="ppool", bufs=1, space="PSUM") as ppool,
        ):
            ptile = ppool.tile([128, 512], dt.float32, tag="pt", name="ptile")
            # ---- constants
            xT = cpool.tile([F0, SHP], dt.bfloat16)
            nc.sync.dma_start(xT[:], p_xT[:])
            W1 = cpool.tile([F0, F1], dt.bfloat16)
            nc.sync.dma_start(W1[:], p_W1[:])
            W2 = cpool.tile([F1, F2], dt.bfloat16)
            nc.sync.dma_start(W2[:], p_W2[:])
            b1b = cpool.tile([128, F1], dt.float32)
            nc.sync.dma_start(b1b[:], p_b1[:])
            b2b = cpool.tile([128, F2], dt.float32)
            nc.sync.dma_start(b2b[:], p_b2[:])
            identb = cpool.tile([128, 128], dt.bfloat16)
            nc.sync.dma_start(identb[:], p_ib[:])
            identf = cpool.tile([128, 128], dt.float32)
            nc.sync.dma_start(identf[:], p_if[:])
            degt = cpool.tile([128, NT], dt.float32)
            nc.sync.dma_start(degt[:], p_deg[:])
            idx_sb = []
            for h in (0, 1):
                isb = cpool.tile([128, half_len[h] // 16], dt.int16,
                                 name=f"isb{h}")
                nc.sync.dma_start(isb[:], p_idx[h][:])
                idx_sb.append(isb)

            recd = cpool.tile([128, NT], dt.float32)
            nc.vector.reciprocal(recd[:], degt[:])
            dinv = cpool.tile([128, NT], dt.float32)
            nc.scalar.activation(dinv[:], recd[:], Act.Sqrt)

            # bf16 tables with zero feature padding [128, NT, 128]
            Tsh = cpool.tile([128, NT * FP], dt.bfloat16)
            nc.vector.memset(Tsh[:], 0.0)
            T2sh = cpool.tile([128, NT * FP], dt.bfloat16)
            nc.vector.memset(T2sh[:], 0.0)

            # ---- head: Tsh = dinv * (x @ W1)
            for t in range(NT):
                psh = hpool.tile([128, F1], dt.float32, tag="hd", name=f"psh{t}")
                nc.tensor.matmul(psh[:], xT[:, BLK * t:BLK * (t + 1)], W1[:],
                                 start=True, stop=True)
                nc.vector.tensor_scalar(
                    Tsh[:, FP * t:FP * t + F1], psh[:], dinv[:, t:t + 1], None,
                    Alu.mult)
            nc.sync.dma_start(
                cc_in[0][:].rearrange("(t p) f -> p t f", p=BLK),
                Tsh.rearrange("p (t f) -> p t f", f=FP)[:])
            nc.gpsimd.collective_compute(
                "AllGather", Alu.bypass,
                ins=[cc_in[0].ap().opt()], outs=[cc_out[0].ap().opt()],
                replica_groups=[list(range(CORES))])

            def do_layer(li, table, own, tail_fn):
                halves = [table[0:HALF, :], table[HALF:2 * HALF, :]]
                poff = 0
                coff = [0, 0]
                for t in range(NT):
                    pagg = apool.tile([128, F1], dt.float32, tag="agg",
                                      name=f"agg{li}_{t}")
                    # self-loop piece opens the accumulation group
                    nc.tensor.matmul(pagg[:], identb[:],
                                     own[:, FP * t:FP * t + F1],
                                     start=True, stop=False)
                    for h in (0, 1):
                        nb = int(B[t, h])
                        st = stpool.tile([128, BMAX, FP], dt.bfloat16,
                                         tag="st", name=f"st{li}_{h}_{t}")
                        nc.gpsimd.dma_gather(
                            st[:, :nb, :], halves[h],
                            idx_sb[h][:, coff[h] // 16:
                                      (coff[h] + nb * BLK) // 16],
                            nb * BLK, nb * BLK, FP, single_packet=False)
                        ssb = spool.tile([128, BMAX * BLK], dt.bfloat16,
                                         tag="ssb", name=f"ss{li}_{h}_{t}")
                        nc.sync.dma_start(
                            ssb[:, :nb * BLK],
                            p_S[:, poff * BLK:(poff + nb) * BLK])
                        for b in range(nb):
                            nc.tensor.matmul(
                                pagg[:], ssb[:, b * BLK:(b + 1) * BLK],
                                st[:, b, 0:F1],
                                start=False,
                                stop=(h == 1 and b == nb - 1))
                        coff[h] += nb * BLK
                        poff += nb
                    tail_fn(t, pagg[:])

            # ---- layer 1
            def tail1(t, pagg):
                e1 = wpool.tile([128, F1], dt.float32, tag="e1", name=f"e1_{t}")
                nc.vector.tensor_scalar(e1[:], pagg, dinv[:, t:t + 1], None,
                                        Alu.mult)
                e2 = wpool.tile([128, F1], dt.float32, tag="e2", name=f"e2_{t}")
                nc.vector.tensor_tensor(out=e2[:], in0=e1[:], in1=b1b[:],
                                        op=Alu.add)
                nc.vector.tensor_scalar(
                    T2sh[:, FP * t:FP * t + F1], e2[:], 0.0, dinv[:, t:t + 1],
                    Alu.max, Alu.mult)

            do_layer(0, cc_out[0], Tsh, tail1)
            nc.sync.dma_start(
                cc_in[1][:].rearrange("(t p) f -> p t f", p=BLK),
                T2sh.rearrange("p (t f) -> p t f", f=FP)[:])
            nc.gpsimd.collective_compute(
                "AllGather", Alu.bypass,
                ins=[cc_in[1].ap().opt()], outs=[cc_out[1].ap().opt()],
                replica_groups=[list(range(CORES))])

            # ---- layer 2
            outsh = cpool.tile([128, NT * F2], dt.float32)

            def tail2(t, pagg):
                cp = wpool.tile([128, F1], dt.float32, tag="cp", name=f"cp_{t}")
                nc.vector.tensor_copy(cp[:], pagg)
                ptr = ptile[0:F1, 0:128]
                nc.tensor.transpose(ptr, cp[:], identf[:])
                aggT = wpool.tile([F1, 128], dt.bfloat16, tag="at", name=f"at_{t}")
                nc.vector.tensor_copy(aggT[:], ptr)
                po = ptile[:, 128:128 + F2]
                nc.tensor.matmul(po, aggT[:], W2[:], start=True, stop=True)
                e3 = wpool.tile([128, F2], dt.float32, tag="e3", name=f"e3_{t}")
                nc.vector.tensor_scalar(e3[:], po, dinv[:, t:t + 1], None,
                                        Alu.mult)
                e4 = wpool.tile([128, F2], dt.float32, tag="e4", name=f"e4_{t}")
                nc.vector.tensor_tensor(out=e4[:], in0=e3[:], in1=b2b[:],
                                        op=Alu.add)
                m = wpool.tile([128, 1], dt.float32, tag="m", name=f"m_{t}")
                nc.vector.tensor_reduce(m[:], e4[:], axis=mybir.AxisListType.X,
                                        op=Alu.max)
                nm = wpool.tile([128, 1], dt.float32, tag="nm", name=f"nm_{t}")
                nc.vector.tensor_scalar(nm[:], m[:], -1.0, None, Alu.mult)
                ex = wpool.tile([128, F2], dt.float32, tag="ex", name=f"ex_{t}")
                nc.scalar.activation(ex[:], e4[:], Act.Exp, bias=nm[:, 0:1])
                sm = wpool.tile([128, 1], dt.float32, tag="sm", name=f"sm_{t}")
                nc.vector.tensor_reduce(sm[:], ex[:], axis=mybir.AxisListType.X,
                                        op=Alu.add)
                lg = wpool.tile([128, 1], dt.float32, tag="lg", name=f"lg_{t}")
                nc.scalar.activation(lg[:], sm[:], Act.Ln)
                nc.vector.tensor_scalar(
                    outsh[:, F2 * t:F2 * (t + 1)], e4[:], m[:, 0:1], lg[:, 0:1],
                    Alu.subtract, Alu.subtract)

            do_layer(1, cc_out[1], T2sh, tail2)
            nc.sync.dma_start(p_out[:], outsh[:])

    nc.finalize()
    return nc


LAST_EXEC_NS = None


def kernel(x, edge_index, W1, b1, W2, b2):
    from concourse.bass_utils import run_bass_kernel_spmd

    x = np.asarray(x, np.float32)
    data, consts, meta = host_prep(x, np.asarray(edge_index), W1, b1, W2, b2)
    nc = build_nc(meta)
    in_maps = []
    for i in range(CORES):
        m = dict(data[i])
        m.update({k: np.ascontiguousarray(v) for k, v in consts.items()})
        in_maps.append(m)
    import os as _os
    trace = bool(int(_os.environ.get("GCN_TRACE", "0")))
    res = run_bass_kernel_spmd(nc, in_maps, core_ids=list(range(CORES)),
                               trace=trace)
    global LAST_EXEC_NS
    LAST_EXEC_NS = res.exec_time_ns
    outs = []
    for i in range(CORES):
        o = res.results[i]["out"]  # [128, NT*F2]
        outs.append(o.reshape(128, NT, F2).transpose(1, 0, 2).reshape(SHP, F2))
    return assemble_output(np.stack(outs))


if __name__ == "__main__":
    import reference
    inputs = {k: np.asarray(v) for k, v in reference.setup_inputs().items()}
    expected = np.asarray(reference.reference(**{k: v for k, v in inputs.items()}))
    data, consts, meta = host_prep(**inputs)
    print("B sum:", int(meta["B"].sum()), "half_len:", meta["half_len"],
          "npieces:", meta["npieces"])
    outs = numpy_device_sim(data, consts, meta)
    got = assemble_output(outs)
    err = np.abs(got - expected)
    rel = err.max() / np.abs(expected).max()
    print(f"numpy-sim max abs err {err.max():.3e}  rel {rel:.3e}")



# revision 14
# speedup vs baseline: 1.7310x; 1.7310x over previous
"""GCN (2-layer) Trainium2 kernel over 8 NeuronCores — v3.

Structure per core (dst-shard = 6250 nodes = 49 tiles of 128):
- head: Tsh = dinv * (x @ W1) as bf16 table rows [node, 128] (64 feats + 64
  garbage pad so gather rows are 256B). Table split in two ROW halves per
  shard (tiles 0:25 / 25:49) -> two AllGathers per layer so aggregation of
  half A overlaps the AllGather of half B.
- aggregation: edges grouped per (dst tile t, src half h) into runs padded
  to a cross-core-uniform B[t,h]*128 slots. Per run: one SWDGE dma_gather
  with TRAILING -1 pad idxs (Q7 ucode trims them) round-robined over 4
  SWDGE queues (num_swdge_queues=4 -> 4 Q7 pairs generate descriptors in
  parallel), scatter one-hots S built on-device by DVE is_equal(iota, dl)
  (dl = per-slot dst lane, 255 for pad slots -> zero column nullifies
  stale gather data), then B matmuls psum[t] += S_piece^T @ G_piece.
- self loops: psum[t] opens with Identity^T @ own[:,t].
- tails: layer1 relu/scale -> T2sh table; layer2 transpose + W2 + bias +
  log_softmax with batched Exp/Ln (3 act-table loads total).
All 49 psum accumulators live in PSUM simultaneously.
"""

import numpy as np

N_NODES = 50000
CORES = 8
SH = 6250          # owned nodes per core
SHP = 6272         # padded shard rows (49*128)
NT = 49            # dst tiles per core
TA = 25            # tiles in table half A
HA = TA * 128      # 3200 rows per core in half A
HB = SHP - HA      # 3072 rows per core in half B
ROWS_A = CORES * HA   # 25600
ROWS_B = CORES * HB   # 24576
F0, F1, F2 = 96, 64, 16
FP = 128           # padded feature width (bf16 row = 256B)
BLK = 128
PAD_LANE = 255.0   # dl value for pad slots -> zero S column


def host_prep(x, edge_index, W1, b1, W2, b2):
    import ml_dtypes
    bf16 = ml_dtypes.bfloat16

    src = np.asarray(edge_index[0], dtype=np.int64)
    dst = np.asarray(edge_index[1], dtype=np.int64)
    deg_full = np.bincount(dst, minlength=N_NODES).astype(np.float32) + 1.0

    # map source node -> (half, table row)
    own = src // SH
    r = src - own * SH
    h_of = (r // 128 >= TA).astype(np.int64)
    lrow_of = np.where(h_of == 0, HA * own + r, HB * own + (r - HA))

    order = np.argsort(dst, kind="stable")
    s_sorted, d_sorted = src[order], dst[order]
    h_sorted, lrow_sorted = h_of[order], lrow_of[order]
    bounds = np.searchsorted(d_sorted, np.arange(0, N_NODES + 1, SH))
    runs = [[[None] * 2 for _ in range(NT)] for _ in range(CORES)]
    counts = np.zeros((CORES, NT, 2), dtype=np.int64)
    for i in range(CORES):
        sl = slice(bounds[i], bounds[i + 1])
        ed = d_sorted[sl] - SH * i          # local dst [0,6250)
        hh = h_sorted[sl]
        lr = lrow_sorted[sl]
        tile_id = ed // BLK
        dl = ed - tile_id * BLK
        key = tile_id * 2 + hh
        # sort by (tile, half, table row) for gather locality
        o = np.lexsort((lr, key))
        key_s, lr_s, dl_s = key[o], lr[o], dl[o]
        kb = np.searchsorted(key_s, np.arange(NT * 2 + 1))
        for t in range(NT):
            for h2 in (0, 1):
                a, b = kb[t * 2 + h2], kb[t * 2 + h2 + 1]
                runs[i][t][h2] = (lr_s[a:b], dl_s[a:b])
                counts[i, t, h2] = b - a

    B = np.maximum(1, -(-counts.max(axis=0) // BLK))      # [NT, 2] uniform
    half_len = [int(B[:, h2].sum()) * BLK for h2 in (0, 1)]
    npieces = int(B.sum())

    data = []
    for i in range(CORES):
        idx_streams = [[], []]
        dl_blob = np.full((BLK, npieces), PAD_LANE, dtype=np.float32)
        poff = 0
        for t in range(NT):
            for h2 in (0, 1):
                lr, dl = runs[i][t][h2]
                nb = int(B[t, h2])
                pad = nb * BLK - len(lr)
                idx_streams[h2].append(
                    np.concatenate([lr, np.full(pad, -1, np.int64)]))
                lanes = np.full(nb * BLK, PAD_LANE, np.float32)
                lanes[:len(dl)] = dl
                dl_blob[:, poff:poff + nb] = lanes.reshape(nb, BLK).T
                poff += nb
        assert poff == npieces

        planes = []
        for h2 in (0, 1):
            si = np.concatenate(idx_streams[h2])
            assert len(si) == half_len[h2]
            pl = si.reshape(-1, 16).T.astype(np.int16)    # [16, len/16]
            planes.append(np.tile(pl, (8, 1)))

        degp = np.ones((BLK, NT), np.float32)
        dshard = deg_full[SH * i:SH * (i + 1)]
        dp = np.concatenate([dshard, np.ones(SHP - SH, np.float32)])
        degp[:, :] = dp.reshape(NT, BLK).T

        xs = np.zeros((F0, SHP), np.float32)
        xs[:, :SH] = np.asarray(x[SH * i:SH * (i + 1)], np.float32).T
        data.append(dict(
            xT=np.ascontiguousarray(xs.astype(bf16)),
            idx0=np.ascontiguousarray(planes[0]),
            idx1=np.ascontiguousarray(planes[1]),
            dl=np.ascontiguousarray(dl_blob.astype(bf16)),
            deg=np.ascontiguousarray(degp),
        ))

    consts = dict(
        W1=np.asarray(W1, np.float32).astype(bf16),
        W2=np.asarray(W2, np.float32).astype(bf16),
        b1b=np.tile(np.asarray(b1, np.float32), (BLK, 1)),
        b2b=np.tile(np.asarray(b2, np.float32), (BLK, 1)),
        identb=np.eye(BLK, dtype=np.float32).astype(bf16),
        identf=np.eye(BLK, dtype=np.float32),
        iota=np.ascontiguousarray(
            np.tile(np.arange(BLK, dtype=np.float32), (BLK, 1)).astype(bf16)),
    )
    meta = dict(B=B, half_len=half_len, npieces=npieces)
    return data, consts, meta


def numpy_device_sim(data, consts, meta):
    """Replay the device algorithm in numpy (validates idx/dl construction)."""
    B = meta["B"]
    dinvs, tables = [], []
    for i in range(CORES):
        d = data[i]
        dinv = 1.0 / np.sqrt(d["deg"])                    # [128, NT]
        dinvs.append(dinv)
        h = d["xT"].astype(np.float32).T @ consts["W1"].astype(np.float32)
        hs = h.reshape(NT, BLK, F1) * dinv.T[:, :, None]
        tables.append(hs.reshape(SHP, F1))

    def make_halves(tbls):
        A = np.concatenate([t[:HA] for t in tbls], 0)     # [25600, 64]
        Bt = np.concatenate([t[HA:] for t in tbls], 0)    # [24576, 64]
        return [A, Bt]

    def layer(halves, i, d, own):
        agg = np.zeros((NT, BLK, F1), np.float32)
        for t in range(NT):
            agg[t] = own[t]                               # self loop piece
        streams = [(d["idx0"] if h2 == 0 else d["idx1"])[:16].T.reshape(-1)
                   for h2 in (0, 1)]
        dlb = d["dl"].astype(np.float32)                  # [128, npieces]
        poff = 0
        soff = [0, 0]
        for t in range(NT):
            for h2 in (0, 1):
                nb = int(B[t, h2])
                for b in range(nb):
                    sl = streams[h2][soff[h2] + b * BLK: soff[h2] + (b + 1) * BLK]
                    sl = sl.astype(np.int64)
                    valid = sl >= 0
                    G = np.zeros((BLK, F1), np.float32)
                    G[valid] = halves[h2][sl[valid]]
                    dl = dlb[:, poff + b]                 # [128]
                    S = (dl[:, None] == np.arange(BLK)[None, :]).astype(np.float32)
                    agg[t] += S.T @ G
                soff[h2] += nb * BLK
                poff += nb
        return agg

    full2 = []
    halves1 = make_halves(tables)
    for i in range(CORES):
        d = data[i]
        own = tables[i].reshape(NT, BLK, F1)
        agg = layer(halves1, i, d, own)
        dinv = dinvs[i]
        t2 = []
        for t in range(NT):
            e = np.maximum(agg[t] * dinv[:, t:t + 1] + consts["b1b"], 0.0) \
                * dinv[:, t:t + 1]
            t2.append(e)
        full2.append(np.stack(t2).reshape(SHP, F1))

    outs = []
    halves2 = make_halves(full2)
    for i in range(CORES):
        d = data[i]
        own2 = full2[i].reshape(NT, BLK, F1)
        agg = layer(halves2, i, d, own2)
        dinv = dinvs[i]
        o = np.zeros((NT, BLK, F2), np.float32)
        for t in range(NT):
            a = agg[t] * dinv[:, t:t + 1]
            z = a @ consts["W2"].astype(np.float32) + consts["b2b"]
            m = z.max(1, keepdims=True)
            ls = z - m - np.log(np.exp(z - m).sum(1, keepdims=True))
            o[t] = ls
        outs.append(o.reshape(SHP, F2))
    return np.stack(outs)


def assemble_output(outs):
    res = np.zeros((N_NODES, F2), np.float32)
    for i in range(CORES):
        res[SH * i:SH * (i + 1)] = outs[i][:SH]
    return res


def build_nc(meta):
    import os
    import concourse.bacc as bacc
    import concourse.tile as tile
    import concourse.mybir as mybir

    dt = mybir.dt
    Alu = mybir.AluOpType
    Act = mybir.ActivationFunctionType
    B = meta["B"]
    half_len = meta["half_len"]
    npieces = meta["npieces"]
    BMAX = int(B.max())
    NQ = int(os.environ.get("GCN_NQ", "4"))
    RR = int(os.environ.get("GCN_RR", "1"))
    MIDAG = int(os.environ.get("GCN_MIDAG", "1"))

    nc = bacc.Bacc(None, target_bir_lowering=False, num_swdge_queues=NQ)
    p_xT = nc.declare_dram_parameter("xT", [F0, SHP], dt.bfloat16, isOutput=False)
    p_idx = [nc.declare_dram_parameter(f"idx{h}", [128, half_len[h] // 16],
                                       dt.int16, isOutput=False) for h in (0, 1)]
    p_dl = nc.declare_dram_parameter("dl", [128, npieces], dt.bfloat16,
                                     isOutput=False)
    p_deg = nc.declare_dram_parameter("deg", [128, NT], dt.float32, isOutput=False)
    p_W1 = nc.declare_dram_parameter("W1", [F0, F1], dt.bfloat16, isOutput=False)
    p_W2 = nc.declare_dram_parameter("W2", [F1, F2], dt.bfloat16, isOutput=False)
    p_b1 = nc.declare_dram_parameter("b1b", [128, F1], dt.float32, isOutput=False)
    p_b2 = nc.declare_dram_parameter("b2b", [128, F2], dt.float32, isOutput=False)
    p_ib = nc.declare_dram_parameter("identb", [128, 128], dt.bfloat16,
                                     isOutput=False)
    p_if = nc.declare_dram_parameter("identf", [128, 128], dt.float32,
                                     isOutput=False)
    p_iota = nc.declare_dram_parameter("iota", [128, 128], dt.bfloat16,
                                       isOutput=False)
    p_out = nc.declare_dram_parameter("out", [128, NT * F2], dt.float32,
                                      isOutput=True)

    # per layer: half-A and half-B collective in/out
    cc_in = [[nc.dram_tensor(f"cc_in{li}{hn}", [n, FP], dt.bfloat16)
              for hn, n in (("a", HA), ("b", HB))] for li in (0, 1)]
    cc_out = [[nc.dram_tensor(f"cc_out{li}{hn}", [n, FP], dt.bfloat16,
                              addr_space="Shared")
               for hn, n in (("a", ROWS_A), ("b", ROWS_B))] for li in (0, 1)]

    with tile.TileContext(nc) as tc:
        with (
            tc.tile_pool(name="cpool", bufs=1) as cpool,
            tc.tile_pool(name="stpool", bufs=8) as stpool,
            tc.tile_pool(name="spool", bufs=8) as spool,
            tc.tile_pool(name="wpool", bufs=6) as wpool,
            tc.tile_pool(name="apool", bufs=7, space="PSUM") as apool,
            tc.tile_pool(name="xpool", bufs=1, space="PSUM") as xpool,
        ):
            # ---- constants
            xT = cpool.tile([F0, SHP], dt.bfloat16)
            nc.sync.dma_start(xT[:], p_xT[:])
            W1 = cpool.tile([F0, F1], dt.bfloat16)
            nc.sync.dma_start(W1[:], p_W1[:])
            W2 = cpool.tile([F1, F2], dt.bfloat16)
            nc.sync.dma_start(W2[:], p_W2[:])
            b1b = cpool.tile([128, F1], dt.float32)
            nc.sync.dma_start(b1b[:], p_b1[:])
            b2b = cpool.tile([128, F2], dt.float32)
            nc.sync.dma_start(b2b[:], p_b2[:])
            identb = cpool.tile([128, 128], dt.bfloat16)
            nc.sync.dma_start(identb[:], p_ib[:])
            identf = cpool.tile([128, 128], dt.float32)
            nc.sync.dma_start(identf[:], p_if[:])
            iota = cpool.tile([128, 128], dt.bfloat16)
            nc.sync.dma_start(iota[:], p_iota[:])
            degt = cpool.tile([128, NT], dt.float32)
            nc.sync.dma_start(degt[:], p_deg[:])
            dlt = cpool.tile([128, npieces], dt.bfloat16)
            nc.sync.dma_start(dlt[:], p_dl[:])
            idx_sb = []
            for h in (0, 1):
                isb = cpool.tile([128, half_len[h] // 16], dt.int16,
                                 name=f"isb{h}")
                nc.sync.dma_start(isb[:], p_idx[h][:])
                idx_sb.append(isb)

            recd = cpool.tile([128, NT], dt.float32)
            nc.vector.reciprocal(recd[:], degt[:])
            dinv = cpool.tile([128, NT], dt.float32)
            nc.scalar.activation(dinv[:], recd[:], Act.Sqrt)

            # tables (bf16, cols 64:128 garbage; gathers read 256B rows but
            # matmuls consume cols 0:64 only)
            Tsh = cpool.tile([128, NT * FP], dt.bfloat16)
            T2sh = cpool.tile([128, NT * FP], dt.bfloat16)
            outsh = cpool.tile([128, NT * F2], dt.float32)
            E4sh = cpool.tile([128, NT * F2], dt.float32)
            Msh = cpool.tile([128, NT], dt.float32)
            SMsh = cpool.tile([128, NT], dt.float32)

            # first-touch memset of the gather tile ring (stale cols 0:64 of
            # skipped pad slots must be finite; S=0 columns nullify them)
            for k in range(8):
                st0 = stpool.tile([128, BMAX, FP], dt.bfloat16, tag="st",
                                  name=f"stz{k}")
                nc.vector.memset(st0[:], 0.0)

            # ---- head: Tsh = dinv * (x @ W1)
            for t in range(NT):
                psh = apool.tile([128, 512], dt.float32, tag="agg",
                                 name=f"hd{t}")[:, 0:F1]
                nc.tensor.matmul(psh, xT[:, BLK * t:BLK * (t + 1)], W1[:],
                                 start=True, stop=True)
                nc.vector.tensor_scalar(
                    Tsh[:, FP * t:FP * t + F1], psh, dinv[:, t:t + 1], None,
                    Alu.mult)

            def send_half(li, table_sh, hh):
                t0, t1 = (0, TA) if hh == 0 else (TA, NT)
                nc.sync.dma_start(
                    cc_in[li][hh][:].rearrange("(t p) f -> p t f", p=BLK),
                    table_sh.rearrange("p (t f) -> p t f", f=FP)[:, t0:t1, :])
                nc.gpsimd.collective_compute(
                    "AllGather", Alu.bypass,
                    ins=[cc_in[li][hh].ap().opt()],
                    outs=[cc_out[li][hh].ap().opt()],
                    replica_groups=[list(range(CORES))])

            send_half(0, Tsh, 0)
            send_half(0, Tsh, 1)

            qctr = [0]

            def do_layer(li, own, tail_fn):
                banks = [apool.tile([128, 512], dt.float32, tag="agg",
                                    name=f"bank{li}_{g}") for g in range(7)]
                paggs = [banks[t // 8][:, F1 * (t % 8):F1 * (t % 8) + F1]
                         for t in range(NT)]
                poffs = np.zeros((NT, 2), np.int64)
                coffs = np.zeros((NT, 2), np.int64)
                po, co = 0, 0
                for t in range(NT):
                    for h in (0, 1):
                        poffs[t, h] = po
                        po += int(B[t, h])
                    coffs[t, 0] = coffs[t, 1] = 0
                co0, co1 = 0, 0
                for t in range(NT):
                    coffs[t, 0] = co0
                    co0 += int(B[t, 0]) * BLK
                    coffs[t, 1] = co1
                    co1 += int(B[t, 1]) * BLK

                for h in (0, 1):
                    for t in range(NT):
                        pagg = paggs[t]
                        if h == 0:
                            # start=True resets the WHOLE psum bank -> only
                            # the first tile of each bank-of-8 may set it
                            nc.tensor.matmul(pagg, identb[:],
                                             own[:, FP * t:FP * t + F1],
                                             start=(t % 8 == 0), stop=False)
                        nb = int(B[t, h])
                        poff = int(poffs[t, h])
                        coff = int(coffs[t, h])
                        st = stpool.tile([128, BMAX, FP], dt.bfloat16,
                                         tag="st", name=f"st{li}_{h}_{t}")
                        nc.gpsimd.dma_gather(
                            st[:, :nb, :], cc_out[li][h][:],
                            idx_sb[h][:, coff // 16:(coff + nb * BLK) // 16],
                            nb * BLK, nb * BLK, FP, single_packet=False,
                            queue_num=(qctr[0] % NQ) if RR else 0)
                        qctr[0] += 1
                        ssb = spool.tile([128, BMAX, BLK], dt.bfloat16,
                                         tag="ssb", name=f"ss{li}_{h}_{t}")
                        nc.vector.tensor_tensor(
                            out=ssb[:, :nb, :],
                            in0=iota[:].unsqueeze(1).broadcast_to([128, nb, 128]),
                            in1=dlt[:, poff:poff + nb].unsqueeze(2)
                                .broadcast_to([128, nb, 128]),
                            op=Alu.is_equal)
                        for b in range(nb):
                            nc.tensor.matmul(
                                pagg, ssb[:, b, :], st[:, b, 0:F1],
                                start=False,
                                stop=(h == 1 and b == nb - 1))
                        if h == 1:
                            tail_fn(t, pagg)

            # ---- layer 1
            def tail1(t, pagg):
                e1 = wpool.tile([128, F1], dt.float32, tag="e1", name=f"e1_{t}")
                nc.vector.tensor_scalar(e1[:], pagg, dinv[:, t:t + 1], None,
                                        Alu.mult)
                e2 = wpool.tile([128, F1], dt.float32, tag="e2", name=f"e2_{t}")
                nc.vector.tensor_tensor(out=e2[:], in0=e1[:], in1=b1b[:],
                                        op=Alu.add)
                e3 = wpool.tile([128, F1], dt.float32, tag="e3", name=f"e3_{t}")
                nc.vector.tensor_scalar(e3[:], e2[:], 0.0, None, Alu.max)
                nc.vector.tensor_scalar(
                    T2sh[:, FP * t:FP * t + F1], e3[:], dinv[:, t:t + 1], None,
                    Alu.mult)
                if MIDAG:
                    if t == TA - 1:
                        send_half(1, T2sh, 0)
                    elif t == NT - 1:
                        send_half(1, T2sh, 1)

            do_layer(0, Tsh, tail1)
            if not MIDAG:
                send_half(1, T2sh, 0)
                send_half(1, T2sh, 1)

            # ---- layer 2
            def tail2(t, pagg):
                cp = wpool.tile([128, F1], dt.float32, tag="cp", name=f"cp_{t}")
                nc.vector.tensor_scalar(cp[:], pagg, dinv[:, t:t + 1], None,
                                        Alu.mult)
                ptile = xpool.tile([128, 512], dt.float32, tag="pt",
                                   name=f"pt_{t}")
                ptr = ptile[0:F1, 0:128]
                nc.tensor.transpose(ptr, cp[:], identf[:])
                aggT = wpool.tile([F1, 128], dt.bfloat16, tag="at", name=f"at_{t}")
                nc.vector.tensor_copy(aggT[:], ptr)
                po = ptile[:, 128:128 + F2]
                nc.tensor.matmul(po, aggT[:], W2[:], start=True, stop=True)
                nc.vector.tensor_tensor(out=E4sh[:, F2 * t:F2 * (t + 1)],
                                        in0=po, in1=b2b[:], op=Alu.add)
                nc.vector.tensor_reduce(Msh[:, t:t + 1],
                                        E4sh[:, F2 * t:F2 * (t + 1)],
                                        axis=mybir.AxisListType.X, op=Alu.max)
                nm = wpool.tile([128, 1], dt.float32, tag="nm", name=f"nm_{t}")
                nc.vector.tensor_scalar(nm[:], Msh[:, t:t + 1], -1.0, None,
                                        Alu.mult)
                ex = wpool.tile([128, F2], dt.float32, tag="ex", name=f"ex_{t}")
                nc.scalar.activation(ex[:], E4sh[:, F2 * t:F2 * (t + 1)],
                                     Act.Exp, bias=nm[:, 0:1])
                nc.vector.tensor_reduce(SMsh[:, t:t + 1], ex[:],
                                        axis=mybir.AxisListType.X, op=Alu.add)

            do_layer(1, T2sh, tail2)

            # batched log + final subtract
            lg = cpool.tile([128, NT], dt.float32)
            nc.scalar.activation(lg[:], SMsh[:], Act.Ln)
            msum = cpool.tile([128, NT], dt.float32)
            nc.vector.tensor_tensor(out=msum[:], in0=Msh[:], in1=lg[:],
                                    op=Alu.add)
            for t in range(NT):
                nc.vector.tensor_scalar(
                    outsh[:, F2 * t:F2 * (t + 1)], E4sh[:, F2 * t:F2 * (t + 1)],
                    msum[:, t:t + 1], None, Alu.subtract)
            nc.sync.dma_start(p_out[:], outsh[:])

    nc.finalize()
    return nc


LAST_EXEC_NS = None


def kernel(x, edge_index, W1, b1, W2, b2):
    from concourse.bass_utils import run_bass_kernel_spmd

    x = np.asarray(x, np.float32)
    data, consts, meta = host_prep(x, np.asarray(edge_index), W1, b1, W2, b2)
    nc = build_nc(meta)
    in_maps = []
    for i in range(CORES):
        m = dict(data[i])
        m.update({k: np.ascontiguousarray(v) for k, v in consts.items()})
        in_maps.append(m)
    import os as _os
    trace = bool(int(_os.environ.get("GCN_TRACE", "0")))
    res = run_bass_kernel_spmd(nc, in_maps, core_ids=list(range(CORES)),
                               trace=trace)
    global LAST_EXEC_NS
    LAST_EXEC_NS = res.exec_time_ns
    outs = []
    for i in range(CORES):
        o = res.results[i]["out"]  # [128, NT*F2]
        outs.append(o.reshape(128, NT, F2).transpose(1, 0, 2).reshape(SHP, F2))
    return assemble_output(np.stack(outs))


if __name__ == "__main__":
    import reference
    inputs = {k: np.asarray(v) for k, v in reference.setup_inputs().items()}
    expected = np.asarray(reference.reference(**{k: v for k, v in inputs.items()}))
    data, consts, meta = host_prep(**inputs)
    print("B sum:", int(meta["B"].sum()), "half_len:", meta["half_len"],
          "npieces:", meta["npieces"])
    outs = numpy_device_sim(data, consts, meta)
    got = assemble_output(outs)
    err = np.abs(got - expected)
    rel = err.max() / np.abs(expected).max()
    print(f"numpy-sim max abs err {err.max():.3e}  rel {rel:.3e}")


# revision 15
# speedup vs baseline: 2.7588x; 1.5938x over previous
"""GCN (2-layer) Trainium2 kernel over 8 NeuronCores — v3.

Structure per core (dst-shard = 6250 nodes = 49 tiles of 128):
- head: Tsh = dinv * (x @ W1) as bf16 table rows [node, 128] (64 feats + 64
  garbage pad so gather rows are 256B). Table split in two ROW halves per
  shard (tiles 0:25 / 25:49) -> two AllGathers per layer so aggregation of
  half A overlaps the AllGather of half B.
- aggregation: edges grouped per (dst tile t, src half h) into runs padded
  to a cross-core-uniform B[t,h]*128 slots. Per run: one SWDGE dma_gather
  with TRAILING -1 pad idxs (Q7 ucode trims them) round-robined over 4
  SWDGE queues (num_swdge_queues=4 -> 4 Q7 pairs generate descriptors in
  parallel), scatter one-hots S built on-device by DVE is_equal(iota, dl)
  (dl = per-slot dst lane, 255 for pad slots -> zero column nullifies
  stale gather data), then B matmuls psum[t] += S_piece^T @ G_piece.
- self loops: psum[t] opens with Identity^T @ own[:,t].
- tails: layer1 relu/scale -> T2sh table; layer2 transpose + W2 + bias +
  log_softmax with batched Exp/Ln (3 act-table loads total).
All 49 psum accumulators live in PSUM simultaneously.
"""

import numpy as np

N_NODES = 50000
CORES = 8
SH = 6250          # owned nodes per core
SHP = 6272         # padded shard rows (49*128)
NT = 49            # dst tiles per core
TA = 25            # tiles in table half A
HA = TA * 128      # 3200 rows per core in half A
HB = SHP - HA      # 3072 rows per core in half B
ROWS_A = CORES * HA   # 25600
ROWS_B = CORES * HB   # 24576
F0, F1, F2 = 96, 64, 16
FP = 128           # padded feature width (bf16 row = 256B)
BLK = 128
PAD_LANE = 255.0   # dl value for pad slots -> zero S column


def host_prep(x, edge_index, W1, b1, W2, b2):
    import ml_dtypes
    bf16 = ml_dtypes.bfloat16

    src = np.asarray(edge_index[0], dtype=np.int64)
    dst = np.asarray(edge_index[1], dtype=np.int64)
    deg_full = np.bincount(dst, minlength=N_NODES).astype(np.float32) + 1.0

    # map source node -> (half, table row)
    own = src // SH
    r = src - own * SH
    h_of = (r // 128 >= TA).astype(np.int64)
    lrow_of = np.where(h_of == 0, HA * own + r, HB * own + (r - HA))

    order = np.argsort(dst, kind="stable")
    s_sorted, d_sorted = src[order], dst[order]
    h_sorted, lrow_sorted = h_of[order], lrow_of[order]
    bounds = np.searchsorted(d_sorted, np.arange(0, N_NODES + 1, SH))
    runs = [[[None] * 2 for _ in range(NT)] for _ in range(CORES)]
    counts = np.zeros((CORES, NT, 2), dtype=np.int64)
    for i in range(CORES):
        sl = slice(bounds[i], bounds[i + 1])
        ed = d_sorted[sl] - SH * i          # local dst [0,6250)
        hh = h_sorted[sl]
        lr = lrow_sorted[sl]
        tile_id = ed // BLK
        dl = ed - tile_id * BLK
        key = tile_id * 2 + hh
        # sort by (tile, half, table row) for gather locality
        o = np.lexsort((lr, key))
        key_s, lr_s, dl_s = key[o], lr[o], dl[o]
        kb = np.searchsorted(key_s, np.arange(NT * 2 + 1))
        for t in range(NT):
            for h2 in (0, 1):
                a, b = kb[t * 2 + h2], kb[t * 2 + h2 + 1]
                runs[i][t][h2] = (lr_s[a:b], dl_s[a:b])
                counts[i, t, h2] = b - a

    B = np.maximum(1, -(-counts.max(axis=0) // BLK))      # [NT, 2] uniform
    half_len = [int(B[:, h2].sum()) * BLK for h2 in (0, 1)]
    npieces = int(B.sum())

    data = []
    for i in range(CORES):
        idx_streams = [[], []]
        dl_blob = np.full((BLK, npieces), PAD_LANE, dtype=np.float32)
        poff = 0
        for t in range(NT):
            for h2 in (0, 1):
                lr, dl = runs[i][t][h2]
                nb = int(B[t, h2])
                pad = nb * BLK - len(lr)
                idx_streams[h2].append(
                    np.concatenate([lr, np.full(pad, -1, np.int64)]))
                lanes = np.full(nb * BLK, PAD_LANE, np.float32)
                lanes[:len(dl)] = dl
                dl_blob[:, poff:poff + nb] = lanes.reshape(nb, BLK).T
                poff += nb
        assert poff == npieces

        planes = []
        for h2 in (0, 1):
            si = np.concatenate(idx_streams[h2])
            assert len(si) == half_len[h2]
            pl = si.reshape(-1, 16).T.astype(np.int16)    # [16, len/16]
            planes.append(np.tile(pl, (8, 1)))

        degp = np.ones((BLK, NT), np.float32)
        dshard = deg_full[SH * i:SH * (i + 1)]
        dp = np.concatenate([dshard, np.ones(SHP - SH, np.float32)])
        degp[:, :] = dp.reshape(NT, BLK).T

        xs = np.zeros((F0, SHP), np.float32)
        xs[:, :SH] = np.asarray(x[SH * i:SH * (i + 1)], np.float32).T
        data.append(dict(
            xT=np.ascontiguousarray(xs.astype(bf16)),
            idx0=np.ascontiguousarray(planes[0]),
            idx1=np.ascontiguousarray(planes[1]),
            dl=np.ascontiguousarray(dl_blob.astype(bf16)),
            deg=np.ascontiguousarray(degp),
        ))

    consts = dict(
        W1=np.asarray(W1, np.float32).astype(bf16),
        W2=np.asarray(W2, np.float32).astype(bf16),
        b1b=np.tile(np.asarray(b1, np.float32), (BLK, 1)),
        b2b=np.tile(np.asarray(b2, np.float32), (BLK, 1)),
        identb=np.eye(BLK, dtype=np.float32).astype(bf16),
        identf=np.eye(BLK, dtype=np.float32),
        iota=np.ascontiguousarray(
            np.tile(np.arange(BLK, dtype=np.float32), (BLK, 1)).astype(bf16)),
    )
    meta = dict(B=B, half_len=half_len, npieces=npieces)
    return data, consts, meta


def numpy_device_sim(data, consts, meta):
    """Replay the device algorithm in numpy (validates idx/dl construction)."""
    B = meta["B"]
    dinvs, tables = [], []
    for i in range(CORES):
        d = data[i]
        dinv = 1.0 / np.sqrt(d["deg"])                    # [128, NT]
        dinvs.append(dinv)
        h = d["xT"].astype(np.float32).T @ consts["W1"].astype(np.float32)
        hs = h.reshape(NT, BLK, F1) * dinv.T[:, :, None]
        tables.append(hs.reshape(SHP, F1))

    def make_halves(tbls):
        A = np.concatenate([t[:HA] for t in tbls], 0)     # [25600, 64]
        Bt = np.concatenate([t[HA:] for t in tbls], 0)    # [24576, 64]
        return [A, Bt]

    def layer(halves, i, d, own):
        agg = np.zeros((NT, BLK, F1), np.float32)
        for t in range(NT):
            agg[t] = own[t]                               # self loop piece
        streams = [(d["idx0"] if h2 == 0 else d["idx1"])[:16].T.reshape(-1)
                   for h2 in (0, 1)]
        dlb = d["dl"].astype(np.float32)                  # [128, npieces]
        poff = 0
        soff = [0, 0]
        for t in range(NT):
            for h2 in (0, 1):
                nb = int(B[t, h2])
                for b in range(nb):
                    sl = streams[h2][soff[h2] + b * BLK: soff[h2] + (b + 1) * BLK]
                    sl = sl.astype(np.int64)
                    valid = sl >= 0
                    G = np.zeros((BLK, F1), np.float32)
                    G[valid] = halves[h2][sl[valid]]
                    dl = dlb[:, poff + b]                 # [128]
                    S = (dl[:, None] == np.arange(BLK)[None, :]).astype(np.float32)
                    agg[t] += S.T @ G
                soff[h2] += nb * BLK
                poff += nb
        return agg

    full2 = []
    halves1 = make_halves(tables)
    for i in range(CORES):
        d = data[i]
        own = tables[i].reshape(NT, BLK, F1)
        agg = layer(halves1, i, d, own)
        dinv = dinvs[i]
        t2 = []
        for t in range(NT):
            e = np.maximum(agg[t] * dinv[:, t:t + 1] + consts["b1b"], 0.0) \
                * dinv[:, t:t + 1]
            t2.append(e)
        full2.append(np.stack(t2).reshape(SHP, F1))

    outs = []
    halves2 = make_halves(full2)
    for i in range(CORES):
        d = data[i]
        own2 = full2[i].reshape(NT, BLK, F1)
        agg = layer(halves2, i, d, own2)
        dinv = dinvs[i]
        o = np.zeros((NT, BLK, F2), np.float32)
        for t in range(NT):
            a = agg[t] * dinv[:, t:t + 1]
            z = a @ consts["W2"].astype(np.float32) + consts["b2b"]
            m = z.max(1, keepdims=True)
            ls = z - m - np.log(np.exp(z - m).sum(1, keepdims=True))
            o[t] = ls
        outs.append(o.reshape(SHP, F2))
    return np.stack(outs)


def assemble_output(outs):
    res = np.zeros((N_NODES, F2), np.float32)
    for i in range(CORES):
        res[SH * i:SH * (i + 1)] = outs[i][:SH]
    return res


def build_nc(meta):
    import os
    import concourse.bacc as bacc
    import concourse.tile as tile
    import concourse.mybir as mybir

    dt = mybir.dt
    Alu = mybir.AluOpType
    Act = mybir.ActivationFunctionType
    B = meta["B"]
    half_len = meta["half_len"]
    npieces = meta["npieces"]
    BMAX = int(B.max())
    NQ = int(os.environ.get("GCN_NQ", "4"))
    RR = int(os.environ.get("GCN_RR", "1"))
    MIDAG = int(os.environ.get("GCN_MIDAG", "1"))

    nc = bacc.Bacc(None, target_bir_lowering=False, num_swdge_queues=NQ)
    p_xT = nc.declare_dram_parameter("xT", [F0, SHP], dt.bfloat16, isOutput=False)
    p_idx = [nc.declare_dram_parameter(f"idx{h}", [128, half_len[h] // 16],
                                       dt.int16, isOutput=False) for h in (0, 1)]
    p_dl = nc.declare_dram_parameter("dl", [128, npieces], dt.bfloat16,
                                     isOutput=False)
    p_deg = nc.declare_dram_parameter("deg", [128, NT], dt.float32, isOutput=False)
    p_W1 = nc.declare_dram_parameter("W1", [F0, F1], dt.bfloat16, isOutput=False)
    p_W2 = nc.declare_dram_parameter("W2", [F1, F2], dt.bfloat16, isOutput=False)
    p_b1 = nc.declare_dram_parameter("b1b", [128, F1], dt.float32, isOutput=False)
    p_b2 = nc.declare_dram_parameter("b2b", [128, F2], dt.float32, isOutput=False)
    p_ib = nc.declare_dram_parameter("identb", [128, 128], dt.bfloat16,
                                     isOutput=False)
    p_if = nc.declare_dram_parameter("identf", [128, 128], dt.float32,
                                     isOutput=False)
    p_iota = nc.declare_dram_parameter("iota", [128, 128], dt.bfloat16,
                                       isOutput=False)
    p_out = nc.declare_dram_parameter("out", [128, NT * F2], dt.float32,
                                      isOutput=True)

    # per layer: half-A and half-B collective in/out
    cc_in = [[nc.dram_tensor(f"cc_in{li}{hn}", [n, FP], dt.bfloat16)
              for hn, n in (("a", HA), ("b", HB))] for li in (0, 1)]
    cc_out = [[nc.dram_tensor(f"cc_out{li}{hn}", [n, FP], dt.bfloat16,
                              addr_space="Shared")
               for hn, n in (("a", ROWS_A), ("b", ROWS_B))] for li in (0, 1)]

    with tile.TileContext(nc) as tc:
        with (
            tc.tile_pool(name="cpool", bufs=1) as cpool,
            tc.tile_pool(name="stpool", bufs=8) as stpool,
            tc.tile_pool(name="spool", bufs=8) as spool,
            tc.tile_pool(name="wpool", bufs=6) as wpool,
            tc.tile_pool(name="apool", bufs=7, space="PSUM") as apool,
            tc.tile_pool(name="xpool", bufs=1, space="PSUM") as xpool,
        ):
            # ---- constants
            xT = cpool.tile([F0, SHP], dt.bfloat16)
            nc.sync.dma_start(xT[:], p_xT[:])
            W1 = cpool.tile([F0, F1], dt.bfloat16)
            nc.sync.dma_start(W1[:], p_W1[:])
            W2 = cpool.tile([F1, F2], dt.bfloat16)
            nc.sync.dma_start(W2[:], p_W2[:])
            b1b = cpool.tile([128, F1], dt.float32)
            nc.sync.dma_start(b1b[:], p_b1[:])
            b2b = cpool.tile([128, F2], dt.float32)
            nc.sync.dma_start(b2b[:], p_b2[:])
            identb = cpool.tile([128, 128], dt.bfloat16)
            nc.sync.dma_start(identb[:], p_ib[:])
            identf = cpool.tile([128, 128], dt.float32)
            nc.sync.dma_start(identf[:], p_if[:])
            iota = cpool.tile([128, 128], dt.bfloat16)
            nc.sync.dma_start(iota[:], p_iota[:])
            degt = cpool.tile([128, NT], dt.float32)
            nc.sync.dma_start(degt[:], p_deg[:])
            dlt = cpool.tile([128, npieces], dt.bfloat16)
            nc.sync.dma_start(dlt[:], p_dl[:])
            idx_sb = []
            for h in (0, 1):
                isb = cpool.tile([128, half_len[h] // 16], dt.int16,
                                 name=f"isb{h}")
                nc.sync.dma_start(isb[:], p_idx[h][:])
                idx_sb.append(isb)

            recd = cpool.tile([128, NT], dt.float32)
            nc.vector.reciprocal(recd[:], degt[:])
            dinv = cpool.tile([128, NT], dt.float32)
            nc.scalar.activation(dinv[:], recd[:], Act.Sqrt)

            # tables (bf16, cols 64:128 garbage; gathers read 256B rows but
            # matmuls consume cols 0:64 only)
            Tsh = cpool.tile([128, NT * FP], dt.bfloat16)
            T2sh = cpool.tile([128, NT * FP], dt.bfloat16)
            outsh = cpool.tile([128, NT * F2], dt.float32)
            E4sh = cpool.tile([128, NT * F2], dt.float32)
            Msh = cpool.tile([128, NT], dt.float32)
            SMsh = cpool.tile([128, NT], dt.float32)

            # first-touch memset of the gather tile ring (stale cols 0:64 of
            # skipped pad slots must be finite; S=0 columns nullify them)
            for k in range(8):
                st0 = stpool.tile([128, BMAX, FP], dt.bfloat16, tag="st",
                                  name=f"stz{k}")
                nc.vector.memset(st0[:], 0.0)

            # ---- head: Tsh = dinv * (x @ W1)
            for t in range(NT):
                psh = apool.tile([128, 512], dt.float32, tag="agg",
                                 name=f"hd{t}")[:, 0:F1]
                nc.tensor.matmul(psh, xT[:, BLK * t:BLK * (t + 1)], W1[:],
                                 start=True, stop=True)
                nc.vector.tensor_scalar(
                    Tsh[:, FP * t:FP * t + F1], psh, dinv[:, t:t + 1], None,
                    Alu.mult)

            def send_half(li, table_sh, hh):
                t0, t1 = (0, TA) if hh == 0 else (TA, NT)
                nc.sync.dma_start(
                    cc_in[li][hh][:].rearrange("(t p) f -> p t f", p=BLK),
                    table_sh.rearrange("p (t f) -> p t f", f=FP)[:, t0:t1, :])
                nc.gpsimd.collective_compute(
                    "AllGather", Alu.bypass,
                    ins=[cc_in[li][hh].ap().opt()],
                    outs=[cc_out[li][hh].ap().opt()],
                    replica_groups=[list(range(CORES))])

            send_half(0, Tsh, 0)
            send_half(0, Tsh, 1)

            qctr = [0]

            def do_layer(li, own, tail_fn):
                banks = [apool.tile([128, 512], dt.float32, tag="agg",
                                    name=f"bank{li}_{g}") for g in range(7)]
                paggs = [banks[t // 8][:, F1 * (t % 8):F1 * (t % 8) + F1]
                         for t in range(NT)]
                poffs = np.zeros((NT, 2), np.int64)
                coffs = np.zeros((NT, 2), np.int64)
                po, co = 0, 0
                for t in range(NT):
                    for h in (0, 1):
                        poffs[t, h] = po
                        po += int(B[t, h])
                    coffs[t, 0] = coffs[t, 1] = 0
                co0, co1 = 0, 0
                for t in range(NT):
                    coffs[t, 0] = co0
                    co0 += int(B[t, 0]) * BLK
                    coffs[t, 1] = co1
                    co1 += int(B[t, 1]) * BLK

                for h in (0, 1):
                    for t in range(NT):
                        pagg = paggs[t]
                        if h == 0:
                            # start=True resets the WHOLE psum bank -> only
                            # the first tile of each bank-of-8 may set it
                            nc.tensor.matmul(pagg, identb[:],
                                             own[:, FP * t:FP * t + F1],
                                             start=(t % 8 == 0), stop=False)
                        nb = int(B[t, h])
                        poff = int(poffs[t, h])
                        coff = int(coffs[t, h])
                        st = stpool.tile([128, BMAX, FP], dt.bfloat16,
                                         tag="st", name=f"st{li}_{h}_{t}")
                        nc.gpsimd.dma_gather(
                            st[:, :nb, :], cc_out[li][h][:],
                            idx_sb[h][:, coff // 16:(coff + nb * BLK) // 16],
                            nb * BLK, nb * BLK, FP, single_packet=False,
                            queue_num=(qctr[0] % NQ) if RR else 0)
                        qctr[0] += 1
                        ssb = spool.tile([128, BMAX, BLK], dt.bfloat16,
                                         tag="ssb", name=f"ss{li}_{h}_{t}")
                        nc.vector.tensor_tensor(
                            out=ssb[:, :nb, :],
                            in0=iota[:].unsqueeze(1).broadcast_to([128, nb, 128]),
                            in1=dlt[:, poff:poff + nb].unsqueeze(2)
                                .broadcast_to([128, nb, 128]),
                            op=Alu.is_equal)
                        for b in range(nb):
                            nc.tensor.matmul(
                                pagg, ssb[:, b, :], st[:, b, 0:F1],
                                start=False,
                                stop=(h == 1 and b == nb - 1))
                        # PE-write + DVE-read of the SAME psum bank is a
                        # fatal HW collision: run tails for a bank of 8
                        # tiles only after the bank's last matmul.
                        if h == 1 and (t % 8 == 7 or t == NT - 1):
                            for t2 in range(t - t % 8, t + 1):
                                tail_fn(t2, paggs[t2])

            # ---- layer 1
            def tail1(t, pagg):
                e1 = wpool.tile([128, F1], dt.float32, tag="e1", name=f"e1_{t}")
                nc.vector.tensor_scalar(e1[:], pagg, dinv[:, t:t + 1], None,
                                        Alu.mult)
                e2 = wpool.tile([128, F1], dt.float32, tag="e2", name=f"e2_{t}")
                nc.vector.tensor_tensor(out=e2[:], in0=e1[:], in1=b1b[:],
                                        op=Alu.add)
                e3 = wpool.tile([128, F1], dt.float32, tag="e3", name=f"e3_{t}")
                nc.vector.tensor_scalar(e3[:], e2[:], 0.0, None, Alu.max)
                nc.vector.tensor_scalar(
                    T2sh[:, FP * t:FP * t + F1], e3[:], dinv[:, t:t + 1], None,
                    Alu.mult)
                if MIDAG:
                    if t == TA - 1:
                        send_half(1, T2sh, 0)
                    elif t == NT - 1:
                        send_half(1, T2sh, 1)

            do_layer(0, Tsh, tail1)
            if not MIDAG:
                send_half(1, T2sh, 0)
                send_half(1, T2sh, 1)

            # ---- layer 2
            def tail2(t, pagg):
                cp = wpool.tile([128, F1], dt.float32, tag="cp", name=f"cp_{t}")
                nc.vector.tensor_scalar(cp[:], pagg, dinv[:, t:t + 1], None,
                                        Alu.mult)
                ptile = xpool.tile([128, 512], dt.float32, tag="pt",
                                   name=f"pt_{t}")
                ptr = ptile[0:F1, 0:128]
                nc.tensor.transpose(ptr, cp[:], identf[:])
                aggT = wpool.tile([F1, 128], dt.bfloat16, tag="at", name=f"at_{t}")
                nc.vector.tensor_copy(aggT[:], ptr)
                po = ptile[:, 128:128 + F2]
                nc.tensor.matmul(po, aggT[:], W2[:], start=True, stop=True)
                nc.vector.tensor_tensor(out=E4sh[:, F2 * t:F2 * (t + 1)],
                                        in0=po, in1=b2b[:], op=Alu.add)
                nc.vector.tensor_reduce(Msh[:, t:t + 1],
                                        E4sh[:, F2 * t:F2 * (t + 1)],
                                        axis=mybir.AxisListType.X, op=Alu.max)
                nm = wpool.tile([128, 1], dt.float32, tag="nm", name=f"nm_{t}")
                nc.vector.tensor_scalar(nm[:], Msh[:, t:t + 1], -1.0, None,
                                        Alu.mult)
                ex = wpool.tile([128, F2], dt.float32, tag="ex", name=f"ex_{t}")
                nc.scalar.activation(ex[:], E4sh[:, F2 * t:F2 * (t + 1)],
                                     Act.Exp, bias=nm[:, 0:1])
                nc.vector.tensor_reduce(SMsh[:, t:t + 1], ex[:],
                                        axis=mybir.AxisListType.X, op=Alu.add)

            do_layer(1, T2sh, tail2)

            # batched log + final subtract
            lg = cpool.tile([128, NT], dt.float32)
            nc.scalar.activation(lg[:], SMsh[:], Act.Ln)
            msum = cpool.tile([128, NT], dt.float32)
            nc.vector.tensor_tensor(out=msum[:], in0=Msh[:], in1=lg[:],
                                    op=Alu.add)
            for t in range(NT):
                nc.vector.tensor_scalar(
                    outsh[:, F2 * t:F2 * (t + 1)], E4sh[:, F2 * t:F2 * (t + 1)],
                    msum[:, t:t + 1], None, Alu.subtract)
            nc.sync.dma_start(p_out[:], outsh[:])

    nc.finalize()
    return nc


LAST_EXEC_NS = None


def kernel(x, edge_index, W1, b1, W2, b2):
    from concourse.bass_utils import run_bass_kernel_spmd

    x = np.asarray(x, np.float32)
    data, consts, meta = host_prep(x, np.asarray(edge_index), W1, b1, W2, b2)
    nc = build_nc(meta)
    in_maps = []
    for i in range(CORES):
        m = dict(data[i])
        m.update({k: np.ascontiguousarray(v) for k, v in consts.items()})
        in_maps.append(m)
    import os as _os
    trace = bool(int(_os.environ.get("GCN_TRACE", "0")))
    res = run_bass_kernel_spmd(nc, in_maps, core_ids=list(range(CORES)),
                               trace=trace)
    global LAST_EXEC_NS
    LAST_EXEC_NS = res.exec_time_ns
    outs = []
    for i in range(CORES):
        o = res.results[i]["out"]  # [128, NT*F2]
        outs.append(o.reshape(128, NT, F2).transpose(1, 0, 2).reshape(SHP, F2))
    return assemble_output(np.stack(outs))


if __name__ == "__main__":
    import reference
    inputs = {k: np.asarray(v) for k, v in reference.setup_inputs().items()}
    expected = np.asarray(reference.reference(**{k: v for k, v in inputs.items()}))
    data, consts, meta = host_prep(**inputs)
    print("B sum:", int(meta["B"].sum()), "half_len:", meta["half_len"],
          "npieces:", meta["npieces"])
    outs = numpy_device_sim(data, consts, meta)
    got = assemble_output(outs)
    err = np.abs(got - expected)
    rel = err.max() / np.abs(expected).max()
    print(f"numpy-sim max abs err {err.max():.3e}  rel {rel:.3e}")
